# revision 16
# baseline (speedup 1.0000x reference)
"""MoE-routed autoencoder (4 experts, 1024->512->128->512->1024) on 8 TRN2 cores.

Strategy (layer-major, two regions):
- Host: sort atoms by expert symbol, deal each expert's atoms evenly across the
  8 cores, pad per-(core,expert) groups to a common per-expert capacity so one
  SPMD program serves all cores. Only the routed expert runs per atom.
- Device: region A streams L1 over all tiles with L2 trailing by 2 tiles and
  L3 trailing by 4; region B runs L4 with the leftover L2/L3s spread between
  its first tiles. Every inter-layer dependency is satisfied tiles in advance,
  so the tensor engine never waits on an eviction. Activations live in
  transposed layout [feat, atoms]; every layer is out[M,N] = W[K,M].T @
  act[K,N] with f16 operands and fp32 PSUM accumulation. Tile widths are
  near-even splits in [256,512] so LDWEIGHTS (~97ns) always hides behind the
  previous matmul.
- PSUM tiles are [128,1024] bank pairs (4KB, bank-aligned); 512-wide tiles
  evict both m-chunks in one instruction. All evictions run on the scalar
  engine: DVE reads of PSUM correlated with a chip-wide ~20% slowdown.
- x/h/z/d live in single whole-phase SBUF slots (fewer pool slots -> shorter
  Tile prologue/epilogue barrier chains, no DMA buffer-recycle waits).
- DMA: first x tile + w1[e0] halves lead the sync HWDGE queue (the first
  m-pair needs only half of w1); remaining x tiles ride the scalar HWDGE
  queue in parallel; weights stream on sync in deadline order (w2+w3 packed
  per expert, w4 double-buffered with the tail pair on gpsimd); y-out is f16
  on gpsimd, the last tile's halves split across scalar+sync. Host upcasts
  y to fp32.
- PE warmup matmuls release the HAM clock gate (1.2 -> 2.4 GHz) while the
  Tile prologue + first DMAs complete; they end right as the first data lands
  so the activity window never lapses (an idle >~3us re-throttles).
"""

import numpy as np

N_CORES = 8

_PROGRAM_CACHE: dict = {}

# test-harness knobs: when _TRACE is set, the SPMD launch requests an NTFF
# profile and the BassKernelResults lands in _LAST["res"].
_TRACE = False
_LAST: dict = {}

_WARMUP_MMS = 16
_EVICT_MODE = "scalar"


def _tile_widths(C):
    """Split capacity C (multiple of 8) into near-even widths in [256, 512]
    so LDWEIGHTS (~97ns) always hides behind each matmul (>=107ns)."""
    if C <= 0:
        return []
    if C <= 512:
        return [C]
    nt = -(-C // 512)
    base = C // nt // 8 * 8
    ws = [base] * nt
    ws[0] += C - base * nt
    return ws


def _plan(dims, tiles):
    """seq of (e, t, co, T, xoff, yoff) + flat x/y sizes."""
    D_IN, H1, LAT, D_OUT, E, _ = dims
    KC1 = D_IN // 128
    MC4 = D_OUT // 128
    xoff, yoff, seq = 0, 0, []
    for e in range(E):
        off, Ts = tiles[e]
        co = off
        for t, T in enumerate(Ts):
            seq.append((e, t, co, T, xoff, yoff))
            co += T
            xoff += 128 * KC1 * T
            yoff += 128 * MC4 * T
    return seq, xoff, yoff


def _build_program(dims, tiles, use_bias, n_bias_cols):
    import concourse.bass as bass  # noqa: F401
    import concourse.tile as tile
    from concourse import bacc, mybir

    D_IN, H1, LAT, D_OUT, E, C_tot = dims
    f32 = mybir.dt.float32
    f16 = mybir.dt.float16
    RELU = mybir.ActivationFunctionType.Relu
    IDENT = mybir.ActivationFunctionType.Identity
    COPY = mybir.ActivationFunctionType.Copy

    KC1, MC1 = D_IN // 128, H1 // 128    # 8, 4
    KC2, MC2 = H1 // 128, LAT // 128     # 4, 1
    KC3, MC3 = LAT // 128, H1 // 128     # 1, 4
    KC4, MC4 = H1 // 128, D_OUT // 128   # 4, 8

    seq, x_total, y_total = _plan(dims, tiles)
    n_tiles = len(seq)
    # column offsets of each tile inside the single x/h/z/d slots
    xco, hco, zco = {}, {}, {}
    xc = hc = zc = 0
    for i, (e, t, co, T, xo, yo) in enumerate(seq):
        xco[i], hco[i], zco[i] = xc, hc, zc
        xc += KC1 * T
        hc += MC1 * T
        zc += MC2 * T

    nc = bacc.Bacc("TRN2", target_bir_lowering=False, debug=False,
                   num_devices=N_CORES)
    xt = nc.dram_tensor("xt", [x_total], f16, kind="ExternalInput").ap()
    # weights m-major: block m = KC chunks of [128,128] each; w2+w3 packed
    w1 = nc.dram_tensor("w1", [E, 128, MC1 * KC1 * 128], f16,
                        kind="ExternalInput").ap()
    w23 = nc.dram_tensor("w23", [E, 128, (MC2 * KC2 + MC3 * KC3) * 128], f16,
                         kind="ExternalInput").ap()
    w4 = nc.dram_tensor("w4", [E, 128, MC4 * KC4 * 128], f16,
                        kind="ExternalInput").ap()
    if use_bias:
        bias = nc.dram_tensor("bias", [128, n_bias_cols], f32,
                              kind="ExternalInput").ap()
    yt = nc.dram_tensor("yt", [y_total], f16, kind="ExternalOutput").ap()

    W23C = (MC2 * KC2 + MC3 * KC3) * 128

    with tile.TileContext(nc) as tc:
        with (
            tc.tile_pool(name="wp1", bufs=3) as wp1,
            tc.tile_pool(name="wp23", bufs=E) as wp23,
            tc.tile_pool(name="ap", bufs=1) as ap,
            tc.tile_pool(name="yp", bufs=3) as yp,
            tc.tile_pool(name="pp", bufs=4, space="PSUM") as pp,
        ):
            if use_bias:
                btile = ap.tile([128, n_bias_cols], f32, tag="bias")
                nc.sync.dma_start(btile[:], bias[:])
                lsz = (H1 + LAT + H1 + D_OUT) // 128

                def bias_ap(e, layer, m):
                    base = e * lsz + (0, MC1, MC1 + MC2,
                                      MC1 + MC2 + MC3)[layer]
                    return btile[:, base + m:base + m + 1]

            evict_flip = [0]

            def evict(out_ap, ps_ap, relu, e=0, layer=0, m=0):
                if use_bias:
                    b = bias_ap(e, layer, m)
                    nc.scalar.activation(out_ap, ps_ap,
                                         RELU if relu else IDENT, bias=b)
                    return
                evict_flip[0] ^= 1
                if _EVICT_MODE == "scalar":
                    nc.scalar.activation(out_ap, ps_ap,
                                         RELU if relu else COPY)
                    return
                if relu:
                    if evict_flip[0]:
                        nc.scalar.activation(out_ap, ps_ap, RELU)
                    else:
                        nc.vector.tensor_scalar_max(out_ap, ps_ap, 0.0)
                else:
                    if evict_flip[0]:
                        nc.scalar.activation(out_ap, ps_ap, COPY)
                    else:
                        nc.vector.tensor_copy(out_ap, ps_ap)

            # single whole-phase activation slots
            xall = ap.tile([128, xc], f16, tag="x")
            hall = ap.tile([128, hc], f16, tag="h")
            zall = ap.tile([128, zc], f16, tag="z")
            dall = ap.tile([128, hc], f16, tag="d")

            # PE warmup: dependency-free matmuls release the HAM clock gate
            # (1.2 -> 2.4 GHz) while the Tile prologue + first DMAs land.
            # 512-wide so few instructions cover the ramp window; memset on
            # gpsimd (its sequencer prologue finishes earliest).
            warm = ap.tile([128, 512], f16, tag="warm")
            nc.gpsimd.memset(warm[:], 0.0)
            wps = pp.tile([128, 2 * 512], f32, tag="ps")
            for _ in range(_WARMUP_MMS):
                nc.tensor.matmul(wps[:, :512], warm[:, :128], warm[:],
                                 start=True, stop=True)

            def wchunk(wt, m, k, KC, base=0):
                c = base + (m * KC + k) * 128
                return wt[:, c:c + 128]

            # bank-pair PSUM allocation: halves at cols [0:T] and [512:512+T]
            def ps_pair():
                return pp.tile([128, 2 * 512], f32, tag="ps", name="pst")

            def layer_tile(wt, wbase, MC, KC, src_ap_fn, dst_ap_fn, T,
                           relu, e, layer, on_chunk=None, singles=False):
                """Emit MC m-chunks (paired per PSUM bank-pair) for one tile.

                singles=True evicts each m-chunk as soon as it finishes (used
                for the first tile, whose weights stream in m-granular pieces).
                """
                for p in range(0, MC, 2):
                    pst = ps_pair()
                    npair = min(2, MC - p)
                    for j in range(npair):
                        m = p + j
                        out = pst[:, j * 512:j * 512 + T]
                        for k in range(KC):
                            nc.tensor.matmul(
                                out, wchunk(wt, m, k, KC, wbase),
                                src_ap_fn(k),
                                start=(k == 0), stop=(k == KC - 1))
                        if singles:
                            evict(dst_ap_fn(m, 1), out, relu, e, layer, m)
                            if on_chunk is not None:
                                on_chunk(m)
                    if singles:
                        continue
                    if npair == 2 and T == 512:
                        evict(dst_ap_fn(p, 2), pst[:, :1024], relu,
                              e, layer, p)
                        if on_chunk is not None:
                            on_chunk(p + 1)
                    else:
                        for j in range(npair):
                            evict(dst_ap_fn(p + j, 1),
                                  pst[:, j * 512:j * 512 + T], relu,
                                  e, layer, p + j)
                            if on_chunk is not None:
                                on_chunk(p + j)

            # -------- region A: L1 stream with L2 trailing by 2 tiles --------
            w1t, w23t, w4t = {}, {}, {}

            def emit_l1(i, e, T):
                xb, hb = xco[i], hco[i]
                layer_tile(
                    w1t[e], 0, MC1, KC1,
                    lambda k, xb=xb, T=T: xall[:, xb + k * T:
                                               xb + (k + 1) * T],
                    lambda m, w, hb=hb, T=T: hall[:, hb + m * T:
                                                  hb + (m + w) * T],
                    T, True, e, 0)

            def emit_l2(i, e, T):
                hb, zb = hco[i], zco[i]
                layer_tile(
                    w23t[e], 0, MC2, KC2,
                    lambda k, hb=hb, T=T: hall[:, hb + k * T:hb + (k + 1) * T],
                    lambda m, w, zb=zb, T=T: zall[:, zb:zb + w * T],
                    T, True, e, 1)

            def emit_l3(i, e, T):
                zb, db = zco[i], hco[i]
                layer_tile(
                    w23t[e], MC2 * KC2 * 128, MC3, KC3,
                    lambda k, zb=zb, T=T: zall[:, zb:zb + T],
                    lambda m, w, db=db, T=T: dall[:, db + m * T:
                                                  db + (m + w) * T],
                    T, True, e, 2)

            def xap_of(i):
                _, _, _, T, xo, _ = seq[i]
                return xt[xo:xo + 128 * KC1 * T].rearrange("(p f) -> p f",
                                                           p=128)

            # ---- upfront DMA plan, global deadline order ----
            # One HWDGE ring alone saturates HBM (~360 GB/s) and the DMA
            # channels drain all rings' descriptors in roughly global issue
            # order, so a transfer completes at ~(bytes issued before it +
            # its own)/360GB/s after stream start (~8.2us).  What matters is
            # issuing bytes in DEADLINE order -- early-issued bulk ahead of
            # startup-critical pieces starves the pipeline head (the v1
            # kernel lost ~7us to exactly that).  Rings only parallelize the
            # ~650ns descriptor-gen: bulk alternates sync/gpsimd; scalar (the
            # eviction engine) gets a single early piece.
            Q = KC1 * 128
            T0 = seq[0][3]
            xhalf = KC1 // 2 * T0
            x0ap = xap_of(0)
            for e in range(min(3, E)):
                w1t[e] = wp1.tile([128, MC1 * KC1 * 128], f16, tag="w1",
                                  name="w1t")
                w23t[e] = wp23.tile([128, W23C], f16, tag="w23",
                                    name="w23t")
            # startup-critical head: x0 halves + w1[e0] m-quarters, in
            # first-use order (tile0's L1 runs m-chunks singly)
            nc.sync.dma_start(xall[:, :xhalf], x0ap[:, :xhalf])
            nc.scalar.dma_start(w1t[0][:, :Q], w1[0][:, :Q])
            nc.gpsimd.dma_start(xall[:, xhalf:KC1 * T0], x0ap[:, xhalf:])
            nc.sync.dma_start(w1t[0][:, Q:2 * Q], w1[0][:, Q:2 * Q])
            nc.gpsimd.dma_start(w1t[0][:, 2 * Q:], w1[0][:, 2 * Q:])
            # the rest sorted by PE-timeline deadline (x[i] at L1[i]; w1[e]
            # halves just before the expert's first L1; w23[e] at its first
            # L2), alternating sync/gpsimd for descriptor-gen overlap
            tpe = 0
            dl_x, dl_l2 = {}, {}
            for i, (e, _t, _co, T, _xo, _yo) in enumerate(seq):
                dl_x[i] = tpe
                tpe += 32 * T
                if i >= 2:
                    dl_l2.setdefault(seq[i - 2][0], tpe)
                    tpe += 4 * seq[i - 2][3]
                if i >= 4:
                    tpe += 4 * seq[i - 4][3]
            first_tile = {}
            for i, s in enumerate(seq):
                first_tile.setdefault(s[0], i)
            items = []
            for i in range(1, n_tiles):
                items.append((dl_x[i], 0, "x", i))
            for e in range(1, min(3, E)):
                i0 = first_tile[e]
                items.append((dl_x[i0] - 2, 0, "w1h0", e))
                items.append((dl_x[i0] - 2, 1, "w1h1", e))
            for e in range(min(3, E)):
                items.append((dl_l2.get(e, tpe), 2, "w23", e))
            items.sort()
            rr = 0
            for _dl, _o, kind, a in items:
                eng = (nc.sync, nc.gpsimd)[rr % 2]
                rr += 1
                if kind == "x":
                    T = seq[a][3]
                    eng.dma_start(xall[:, xco[a]:xco[a] + KC1 * T], xap_of(a))
                elif kind == "w1h0":
                    eng.dma_start(w1t[a][:, :2 * Q], w1[a][:, :2 * Q])
                elif kind == "w1h1":
                    eng.dma_start(w1t[a][:, 2 * Q:], w1[a][:, 2 * Q:])
                else:
                    eng.dma_start(w23t[a][:], w23[a])

            def emit_dma_for_expert(e):
                if e < 3:
                    return
                w1t[e] = wp1.tile([128, MC1 * KC1 * 128], f16, tag="w1",
                                  name="w1t")
                nc.sync.dma_start(w1t[e][:], w1[e])
                w23t[e] = wp23.tile([128, W23C], f16, tag="w23", name="w23t")
                nc.sync.dma_start(w23t[e][:], w23[e])

            cur_e = -1
            for i, (e, t, co, T, xo, yo) in enumerate(seq):
                if e != cur_e:
                    emit_dma_for_expert(e)
                    cur_e = e
                if i == 0:
                    xb, hb = xco[0], hco[0]
                    layer_tile(
                        w1t[e], 0, MC1, KC1,
                        lambda k, xb=xb, T=T: xall[:, xb + k * T:
                                                   xb + (k + 1) * T],
                        lambda m, w, hb=hb, T=T: hall[:, hb + m * T:
                                                      hb + (m + w) * T],
                        T, True, e, 0, singles=True)
                    continue
                emit_l1(i, e, T)
                if i >= 2:
                    ee, tt, cco, TT, _, _ = seq[i - 2]
                    emit_l2(i - 2, ee, TT)
                if i >= 4:
                    ee, tt, cco, TT, _, _ = seq[i - 4]
                    emit_l3(i - 4, ee, TT)
            for e in range(min(2, E)):
                w4t[e] = wp1.tile([128, MC4 * KC4 * 128], f16, tag="w1",
                                  name="w4t")
                nc.gpsimd.dma_start(w4t[e][:], w4[e])
            catchup = ([("l2", i) for i in range(max(0, n_tiles - 2),
                                                  n_tiles)] +
                       [("l3", i) for i in range(max(0, n_tiles - 4),
                                                 n_tiles)])

            # -------- region B: L3 leading L4 by 2 tiles --------
            yring = [0]
            YENGS = (nc.gpsimd, nc.sync)

            def emit_l4(si, e, T):
                last = si == n_tiles - 1
                yo = seq[si][5]
                db = hco[si]
                yap = yt[yo:yo + 128 * MC4 * T].rearrange("(p f) -> p f",
                                                          p=128)
                half = MC4 // 2
                ytl0 = yp.tile([128, half * T], f16, tag="y", name="ytl0")
                ytl1 = yp.tile([128, half * T], f16, tag="y", name="ytl1")

                def on_chunk(m, ytl0=ytl0, ytl1=ytl1, yap=yap, half=half,
                             T=T, last=last, si=si):
                    if last:
                        # store per evicted pair across three idle rings; the
                        # final pair goes out in two half-pair stores so the
                        # tail waits on ~64KB, not 512KB
                        if m % 2 == 1:
                            p = m - 1
                            ytl = ytl0 if p < half else ytl1
                            mm = p % half
                            if p + 2 < MC4:
                                eng = YENGS[(p // 2) % 2]
                                eng.dma_start(yap[:, p * T:(p + 2) * T],
                                              ytl[:, mm * T:(mm + 2) * T])
                            else:
                                nc.sync.dma_start(
                                    yap[:, p * T:(p + 1) * T],
                                    ytl[:, mm * T:(mm + 1) * T])
                                nc.gpsimd.dma_start(
                                    yap[:, (p + 1) * T:(p + 2) * T],
                                    ytl[:, (mm + 1) * T:(mm + 2) * T])
                        return
                    # per-tile halves rotate across the two idle rings
                    if m == half - 1:
                        YENGS[yring[0] % 2].dma_start(yap[:, :half * T],
                                                      ytl0[:])
                        yring[0] += 1
                    elif m == MC4 - 1:
                        YENGS[yring[0] % 2].dma_start(yap[:, half * T:],
                                                      ytl1[:])
                        yring[0] += 1

                def ydst(m, w, ytl0=ytl0, ytl1=ytl1, half=half, T=T):
                    ytl = ytl0 if m < half else ytl1
                    mm = m % half
                    return ytl[:, mm * T:(mm + w) * T]

                layer_tile(w4t[e], 0, MC4, KC4,
                           lambda k, db=db, T=T: dall[:, db + k * T:
                                                      db + (k + 1) * T],
                           ydst, T, False, e, 3, on_chunk=on_chunk)

                # stream the next w4 once this expert's tiles are done
                if (si + 1 == n_tiles or seq[si + 1][0] != e):
                    enext = e + 2
                    if enext < E and tiles[enext][1]:
                        w4t[enext] = wp1.tile([128, MC4 * KC4 * 128], f16,
                                              tag="w1", name="w4t")
                        eng = nc.sync if enext == 2 else nc.gpsimd
                        eng.dma_start(w4t[enext][:], w4[enext])

            # leftover L2/L3s are spread between the first L4 tiles
            # (independent experts, ample scalar slack)
            for i, (e, t, co, T, xo, yo) in enumerate(seq):
                emit_l4(i, e, T)
                if catchup:
                    what, j = catchup.pop(0)
                    ee, tt, cco, TT, _, _ = seq[j]
                    (emit_l2 if what == "l2" else emit_l3)(j, ee, TT)

    nc.compile()
    return nc


def kernel(**inputs) -> np.ndarray:
    from concourse.bass_utils import run_bass_kernel_spmd

    X = np.ascontiguousarray(inputs["X"], dtype=np.float32)
    sym_ids = np.asarray(inputs["sym_ids"]).astype(np.int64).ravel()
    We = [inputs["We1"], inputs["We2"], inputs["Wd1"], inputs["Wd2"]]
    be = [np.asarray(inputs["be1"], dtype=np.float32),
          np.asarray(inputs["be2"], dtype=np.float32),
          np.asarray(inputs["bd1"], dtype=np.float32),
          np.asarray(inputs["bd2"], dtype=np.float32)]

    N, D_IN = X.shape
    E, _, H1 = We[0].shape
    LAT = We[1].shape[2]
    D_OUT = We[3].shape[2]
    KC1 = D_IN // 128
    MC4 = D_OUT // 128
    use_bias = any(np.any(b) for b in be)

    # ---- host routing: per-expert, per-core index assignment ----
    core_idx = [[None] * E for _ in range(N_CORES)]
    C_e = [0] * E
    for e in range(E):
        idx = np.flatnonzero(sym_ids == e)
        n = len(idx)
        base, rem = divmod(n, N_CORES)
        s = 0
        for c in range(N_CORES):
            cnt = base + (1 if c < rem else 0)
            core_idx[c][e] = idx[s:s + cnt]
            s += cnt
        C_e[e] = -(-(base + (1 if rem else 0)) // 8) * 8  # pad to mult of 8

    # per-expert tiling: widths in [128,512]; the first expert leads with its
    # narrowest tile (fast start on minimal data), the last expert ends with
    # its narrowest (minimal exposed tail)
    tiles = []
    off = 0
    first_e = next((e for e in range(E) if C_e[e]), 0)
    last_e = next((e for e in reversed(range(E)) if C_e[e]), E - 1)
    for e in range(E):
        if e == last_e and 1024 < C_e[e] <= 1280:
            # end on a 256-wide tile: shortest exposed tail chain
            ws = [512, C_e[e] - 768, 256]
        elif e == first_e and C_e[e] > 512 + 256:
            # lead with a 256-wide tile: minimal startup-critical x DMA
            ws = [256] + sorted(_tile_widths(C_e[e] - 256))
        else:
            ws = _tile_widths(C_e[e])
            if e == first_e:
                ws.sort()
            if e == last_e:
                ws.sort(reverse=True)
        tiles.append((off, tuple(ws)))
        off += sum(ws)
    C_tot = off

    # ---- build / fetch compiled program ----
    dims = (D_IN, H1, LAT, D_OUT, E, C_tot)
    n_bias_cols = E * (H1 + LAT + H1 + D_OUT) // 128
    key = (dims, tuple(tiles), use_bias, _EVICT_MODE)
    nc = _PROGRAM_CACHE.get(key)
    if nc is None:
        nc = _build_program(dims, tiles, use_bias, n_bias_cols)
        _PROGRAM_CACHE[key] = nc

    # ---- prepare inputs ----
    XrT = np.ascontiguousarray(X.astype(np.float16).T)       # [D_IN, N]
    XrT_z = np.concatenate(
        [XrT, np.zeros((D_IN, 1), np.float16)], axis=1)      # pad col = N

    # weights in device layout: [E, 128, MC*KC*128] (m-major blocks)
    def wdev(w, kc, mw):
        mc = mw // 128
        return np.ascontiguousarray(
            w.astype(np.float32).astype(np.float16)
            .reshape(E, kc, 128, mc, 128).transpose(0, 2, 3, 1, 4)
            .reshape(E, 128, mc * kc * 128))

    Wd1 = wdev(We[0], KC1, H1)
    Wd2 = wdev(We[1], H1 // 128, LAT)
    Wd3 = wdev(We[2], LAT // 128, H1)
    Wd23 = np.concatenate([Wd2, Wd3], axis=2)
    Wd4 = wdev(We[3], H1 // 128, D_OUT)

    seq, x_total, y_total = _plan(dims, tiles)

    bias_h = None
    if use_bias:
        bias_h = np.zeros((128, n_bias_cols), np.float32)
        col = 0
        for e in range(E):
            for b in (be[0][e], be[1][e], be[2][e], be[3][e]):
                for mch in range(len(b) // 128):
                    bias_h[:, col] = b[mch * 128:(mch + 1) * 128]
                    col += 1

    perms = []
    in_maps = []
    for c in range(N_CORES):
        perm = np.full(C_tot, N, dtype=np.int64)
        for e in range(E):
            o = tiles[e][0]
            idx = core_idx[c][e]
            perm[o:o + len(idx)] = idx
        perms.append(perm)
        g3 = XrT_z[:, perm].reshape(KC1, 128, C_tot)
        xflat = np.empty(x_total, dtype=np.float16)
        for e, t, co, T, xo, yo in seq:
            xflat[xo:xo + 128 * KC1 * T] = (
                g3[:, :, co:co + T].transpose(1, 0, 2).reshape(-1))
        m = {"xt": xflat, "w1": Wd1, "w23": Wd23, "w4": Wd4}
        if use_bias:
            m["bias"] = bias_h
        in_maps.append(m)

    res = run_bass_kernel_spmd(nc, in_maps, core_ids=list(range(N_CORES)),
                               trace=_TRACE)
    _LAST["res"] = res

    # ---- unshard ----
    Y = np.empty((N, D_OUT), dtype=np.float32)
    for c in range(N_CORES):
        yflat = res.results[c]["yt"]
        ytc = np.empty((D_OUT, C_tot), dtype=np.float32)
        for e, t, co, T, xo, yo in seq:
            ytc[:, co:co + T] = (
                yflat[yo:yo + 128 * MC4 * T].astype(np.float32)
                .reshape(128, MC4, T).transpose(1, 0, 2).reshape(D_OUT, T))
        perm = perms[c]
        valid = perm != N
        Y[perm[valid]] = ytc.T[valid]
    return Y



# revision 22
# speedup vs baseline: 1.0299x; 1.0299x over previous
"""MoE-routed autoencoder (4 experts, 1024->512->128->512->1024) on 8 TRN2 cores.

Strategy (layer-major, two regions):
- Host: sort atoms by expert symbol, deal each expert's atoms evenly across the
  8 cores, pad per-(core,expert) groups to a common per-expert capacity so one
  SPMD program serves all cores. Only the routed expert runs per atom.
- Device: region A streams L1 over all tiles with L2 trailing by 2 tiles and
  L3 trailing by 4; region B runs L4 with the leftover L2/L3s spread between
  its first tiles. Every inter-layer dependency is satisfied tiles in advance,
  so the tensor engine never waits on an eviction. Activations live in
  transposed layout [feat, atoms]; every layer is out[M,N] = W[K,M].T @
  act[K,N] with f16 operands and fp32 PSUM accumulation. Tile widths are
  near-even splits in [256,512] so LDWEIGHTS (~97ns) always hides behind the
  previous matmul.
- PSUM tiles are [128,1024] bank pairs (4KB, bank-aligned); 512-wide tiles
  evict both m-chunks in one instruction. All evictions run on the scalar
  engine: DVE reads of PSUM correlated with a chip-wide ~20% slowdown.
- x/h/z/d live in single whole-phase SBUF slots (fewer pool slots -> shorter
  Tile prologue/epilogue barrier chains, no DMA buffer-recycle waits).
- DMA: first x tile + w1[e0] halves lead the sync HWDGE queue (the first
  m-pair needs only half of w1); remaining x tiles ride the scalar HWDGE
  queue in parallel; weights stream on sync in deadline order (w2+w3 packed
  per expert, w4 double-buffered with the tail pair on gpsimd); y-out is f16
  on gpsimd, the last tile's halves split across scalar+sync. Host upcasts
  y to fp32.
- PE warmup matmuls release the HAM clock gate (1.2 -> 2.4 GHz) while the
  Tile prologue + first DMAs complete; they end right as the first data lands
  so the activity window never lapses (an idle >~3us re-throttles).
"""

import numpy as np

N_CORES = 8

_PROGRAM_CACHE: dict = {}

# test-harness knobs: when _TRACE is set, the SPMD launch requests an NTFF
# profile and the BassKernelResults lands in _LAST["res"].
_TRACE = False
_LAST: dict = {}

_WARMUP_MMS = 15
_EVICT_MODE = "scalar"


def _tile_widths(C):
    """Split capacity C (multiple of 8) into near-even widths in [256, 512]
    so LDWEIGHTS (~97ns) always hides behind each matmul (>=107ns)."""
    if C <= 0:
        return []
    if C <= 512:
        return [C]
    nt = -(-C // 512)
    base = C // nt // 8 * 8
    ws = [base] * nt
    ws[0] += C - base * nt
    return ws


def _plan(dims, tiles):
    """seq of (e, t, co, T, xoff, yoff) + flat x/y sizes."""
    D_IN, H1, LAT, D_OUT, E, _ = dims
    KC1 = D_IN // 128
    MC4 = D_OUT // 128
    xoff, yoff, seq = 0, 0, []
    for e in range(E):
        off, Ts = tiles[e]
        co = off
        for t, T in enumerate(Ts):
            seq.append((e, t, co, T, xoff, yoff))
            co += T
            xoff += 128 * KC1 * T
            yoff += 128 * MC4 * T
    return seq, xoff, yoff


def _build_program(dims, tiles, use_bias, n_bias_cols):
    import concourse.bass as bass  # noqa: F401
    import concourse.tile as tile
    from concourse import bacc, mybir

    D_IN, H1, LAT, D_OUT, E, C_tot = dims
    f32 = mybir.dt.float32
    f16 = mybir.dt.float16
    RELU = mybir.ActivationFunctionType.Relu
    IDENT = mybir.ActivationFunctionType.Identity
    COPY = mybir.ActivationFunctionType.Copy

    KC1, MC1 = D_IN // 128, H1 // 128    # 8, 4
    KC2, MC2 = H1 // 128, LAT // 128     # 4, 1
    KC3, MC3 = LAT // 128, H1 // 128     # 1, 4
    KC4, MC4 = H1 // 128, D_OUT // 128   # 4, 8

    seq, x_total, y_total = _plan(dims, tiles)
    n_tiles = len(seq)
    # column offsets of each tile inside the single x/h/z/d slots
    xco, hco, zco = {}, {}, {}
    xc = hc = zc = 0
    for i, (e, t, co, T, xo, yo) in enumerate(seq):
        xco[i], hco[i], zco[i] = xc, hc, zc
        xc += KC1 * T
        hc += MC1 * T
        zc += MC2 * T

    nc = bacc.Bacc("TRN2", target_bir_lowering=False, debug=False,
                   num_devices=N_CORES)
    xt = nc.dram_tensor("xt", [x_total], f16, kind="ExternalInput").ap()
    # weights m-major: block m = KC chunks of [128,128] each; w2+w3 packed
    w1 = nc.dram_tensor("w1", [E, 128, MC1 * KC1 * 128], f16,
                        kind="ExternalInput").ap()
    w23 = nc.dram_tensor("w23", [E, 128, (MC2 * KC2 + MC3 * KC3) * 128], f16,
                         kind="ExternalInput").ap()
    w4 = nc.dram_tensor("w4", [E, 128, MC4 * KC4 * 128], f16,
                        kind="ExternalInput").ap()
    if use_bias:
        bias = nc.dram_tensor("bias", [128, n_bias_cols], f32,
                              kind="ExternalInput").ap()
    yt = nc.dram_tensor("yt", [y_total], f16, kind="ExternalOutput").ap()

    W23C = (MC2 * KC2 + MC3 * KC3) * 128

    with tile.TileContext(nc) as tc:
        with (
            tc.tile_pool(name="wp1", bufs=3) as wp1,
            tc.tile_pool(name="wp23", bufs=E) as wp23,
            tc.tile_pool(name="ap", bufs=1) as ap,
            tc.tile_pool(name="yp", bufs=3) as yp,
            tc.tile_pool(name="pp", bufs=4, space="PSUM") as pp,
        ):
            if use_bias:
                btile = ap.tile([128, n_bias_cols], f32, tag="bias")
                nc.sync.dma_start(btile[:], bias[:])
                lsz = (H1 + LAT + H1 + D_OUT) // 128

                def bias_ap(e, layer, m):
                    base = e * lsz + (0, MC1, MC1 + MC2,
                                      MC1 + MC2 + MC3)[layer]
                    return btile[:, base + m:base + m + 1]

            evict_flip = [0]

            def evict(out_ap, ps_ap, relu, e=0, layer=0, m=0):
                if use_bias:
                    b = bias_ap(e, layer, m)
                    nc.scalar.activation(out_ap, ps_ap,
                                         RELU if relu else IDENT, bias=b)
                    return
                evict_flip[0] ^= 1
                if _EVICT_MODE == "scalar":
                    nc.scalar.activation(out_ap, ps_ap,
                                         RELU if relu else COPY)
                    return
                if relu:
                    if evict_flip[0]:
                        nc.scalar.activation(out_ap, ps_ap, RELU)
                    else:
                        nc.vector.tensor_scalar_max(out_ap, ps_ap, 0.0)
                else:
                    if evict_flip[0]:
                        nc.scalar.activation(out_ap, ps_ap, COPY)
                    else:
                        nc.vector.tensor_copy(out_ap, ps_ap)

            # single whole-phase activation slots
            xall = ap.tile([128, xc], f16, tag="x")
            hall = ap.tile([128, hc], f16, tag="h")
            zall = ap.tile([128, zc], f16, tag="z")
            dall = ap.tile([128, hc], f16, tag="d")

            # PE warmup: dependency-free matmuls release the HAM clock gate
            # (1.2 -> 2.4 GHz) while the Tile prologue + first DMAs land.
            # 512-wide so few instructions cover the ramp window; memset on
            # gpsimd (its sequencer prologue finishes earliest).
            warm = ap.tile([128, 512], f16, tag="warm")
            nc.gpsimd.memset(warm[:], 0.0)
            wps = pp.tile([128, 2 * 512], f32, tag="ps")
            for _ in range(_WARMUP_MMS):
                nc.tensor.matmul(wps[:, :512], warm[:, :128], warm[:],
                                 start=True, stop=True)

            def wchunk(wt, m, k, KC, base=0):
                c = base + (m * KC + k) * 128
                return wt[:, c:c + 128]

            # bank-pair PSUM allocation: halves at cols [0:T] and [512:512+T]
            def ps_pair():
                return pp.tile([128, 2 * 512], f32, tag="ps", name="pst")

            def layer_tile(wt, wbase, MC, KC, src_ap_fn, dst_ap_fn, T,
                           relu, e, layer, on_chunk=None, singles=False):
                """Emit MC m-chunks (paired per PSUM bank-pair) for one tile.

                singles=True evicts each m-chunk as soon as it finishes (used
                for the first tile, whose weights stream in m-granular pieces).
                """
                for p in range(0, MC, 2):
                    pst = ps_pair()
                    npair = min(2, MC - p)
                    for j in range(npair):
                        m = p + j
                        out = pst[:, j * 512:j * 512 + T]
                        for k in range(KC):
                            nc.tensor.matmul(
                                out, wchunk(wt, m, k, KC, wbase),
                                src_ap_fn(k),
                                start=(k == 0), stop=(k == KC - 1))
                        if singles:
                            evict(dst_ap_fn(m, 1), out, relu, e, layer, m)
                            if on_chunk is not None:
                                on_chunk(m)
                    if singles:
                        continue
                    if npair == 2 and T == 512:
                        evict(dst_ap_fn(p, 2), pst[:, :1024], relu,
                              e, layer, p)
                        if on_chunk is not None:
                            on_chunk(p + 1)
                    else:
                        for j in range(npair):
                            evict(dst_ap_fn(p + j, 1),
                                  pst[:, j * 512:j * 512 + T], relu,
                                  e, layer, p + j)
                            if on_chunk is not None:
                                on_chunk(p + j)

            # -------- region A: L1 stream with L2 trailing by 2 tiles --------
            w1t, w23t, w4t = {}, {}, {}

            def emit_l1(i, e, T):
                xb, hb = xco[i], hco[i]
                layer_tile(
                    w1t[e], 0, MC1, KC1,
                    lambda k, xb=xb, T=T: xall[:, xb + k * T:
                                               xb + (k + 1) * T],
                    lambda m, w, hb=hb, T=T: hall[:, hb + m * T:
                                                  hb + (m + w) * T],
                    T, True, e, 0)

            def emit_l2(i, e, T):
                hb, zb = hco[i], zco[i]
                layer_tile(
                    w23t[e], 0, MC2, KC2,
                    lambda k, hb=hb, T=T: hall[:, hb + k * T:hb + (k + 1) * T],
                    lambda m, w, zb=zb, T=T: zall[:, zb:zb + w * T],
                    T, True, e, 1)

            def emit_l3(i, e, T):
                zb, db = zco[i], hco[i]
                layer_tile(
                    w23t[e], MC2 * KC2 * 128, MC3, KC3,
                    lambda k, zb=zb, T=T: zall[:, zb:zb + T],
                    lambda m, w, db=db, T=T: dall[:, db + m * T:
                                                  db + (m + w) * T],
                    T, True, e, 2)

            def xap_of(i):
                _, _, _, T, xo, _ = seq[i]
                return xt[xo:xo + 128 * KC1 * T].rearrange("(p f) -> p f",
                                                           p=128)

            # ---- upfront DMA plan, global deadline order ----
            # One HWDGE ring alone saturates HBM (~360 GB/s) and the DMA
            # channels drain all rings' descriptors in roughly global issue
            # order, so a transfer completes at ~(bytes issued before it +
            # its own)/360GB/s after stream start (~8.2us).  What matters is
            # issuing bytes in DEADLINE order -- early-issued bulk ahead of
            # startup-critical pieces starves the pipeline head (the v1
            # kernel lost ~7us to exactly that).  Rings only parallelize the
            # ~650ns descriptor-gen: bulk alternates sync/gpsimd; scalar (the
            # eviction engine) gets a single early piece.
            Q = KC1 * 128
            T0 = seq[0][3]
            xhalf = KC1 // 2 * T0
            x0ap = xap_of(0)
            for e in range(min(3, E)):
                w1t[e] = wp1.tile([128, MC1 * KC1 * 128], f16, tag="w1",
                                  name="w1t")
                w23t[e] = wp23.tile([128, W23C], f16, tag="w23",
                                    name="w23t")
            # startup-critical head: x0 halves + w1[e0] m-quarters + x1
            # halves, in first-use order (tile0's L1 runs m-chunks singly).
            # Only the two HWDGE rings (sync/scalar) move bulk: the gpsimd
            # software-DGE ring has ~4us latency, ~90GB/s, and its slow
            # descriptors block later HWDGE descriptors in the shared channel
            # FIFOs.  scalar's descriptor-gens all land before its first
            # eviction (~11us).
            nc.sync.dma_start(xall[:, :xhalf], x0ap[:, :xhalf])
            nc.scalar.dma_start(xall[:, xhalf:KC1 * T0], x0ap[:, xhalf:])
            nc.sync.dma_start(w1t[0][:, :Q], w1[0][:, :Q])
            nc.scalar.dma_start(w1t[0][:, Q:2 * Q], w1[0][:, Q:2 * Q])
            nc.sync.dma_start(w1t[0][:, 2 * Q:3 * Q], w1[0][:, 2 * Q:3 * Q])
            nc.scalar.dma_start(w1t[0][:, 3 * Q:], w1[0][:, 3 * Q:])
            if n_tiles > 1:
                T1 = seq[1][3]
                x1h = KC1 // 2 * T1
                x1ap = xap_of(1)
                c1 = xco[1]
                nc.sync.dma_start(xall[:, c1:c1 + x1h], x1ap[:, :x1h])
                nc.scalar.dma_start(xall[:, c1 + x1h:c1 + KC1 * T1],
                                    x1ap[:, x1h:])
            # the rest sorted by PE-timeline deadline (x[i] at L1[i]; w1[e]
            # halves just before the expert's first L1; w23[e] at its first
            # L2), alternating sync/gpsimd for descriptor-gen overlap
            tpe = 0
            dl_x, dl_l2 = {}, {}
            for i, (e, _t, _co, T, _xo, _yo) in enumerate(seq):
                dl_x[i] = tpe
                tpe += 32 * T
                if i >= 2:
                    dl_l2.setdefault(seq[i - 2][0], tpe)
                    tpe += 4 * seq[i - 2][3]
                if i >= 4:
                    tpe += 4 * seq[i - 4][3]
            first_tile = {}
            for i, s in enumerate(seq):
                first_tile.setdefault(s[0], i)
            items = []
            for i in range(2, n_tiles):
                items.append((dl_x[i], 0, "x", i))
            for e in range(1, min(3, E)):
                i0 = first_tile[e]
                items.append((dl_x[i0] - 2, 0, "w1h0", e))
                items.append((dl_x[i0] - 2, 1, "w1h1", e))
            for e in range(min(3, E)):
                items.append((dl_l2.get(e, tpe), 2, "w23", e))
            items.sort()
            for _dl, _o, kind, a in items:
                eng = nc.sync
                if kind == "x":
                    T = seq[a][3]
                    eng.dma_start(xall[:, xco[a]:xco[a] + KC1 * T], xap_of(a))
                elif kind == "w1h0":
                    eng.dma_start(w1t[a][:, :2 * Q], w1[a][:, :2 * Q])
                elif kind == "w1h1":
                    eng.dma_start(w1t[a][:, 2 * Q:], w1[a][:, 2 * Q:])
                else:
                    eng.dma_start(w23t[a][:], w23[a])

            def emit_dma_for_expert(e):
                if e < 3:
                    return
                w1t[e] = wp1.tile([128, MC1 * KC1 * 128], f16, tag="w1",
                                  name="w1t")
                nc.sync.dma_start(w1t[e][:], w1[e])
                w23t[e] = wp23.tile([128, W23C], f16, tag="w23", name="w23t")
                nc.sync.dma_start(w23t[e][:], w23[e])

            cur_e = -1
            for i, (e, t, co, T, xo, yo) in enumerate(seq):
                if e != cur_e:
                    emit_dma_for_expert(e)
                    cur_e = e
                if i == 0:
                    xb, hb = xco[0], hco[0]
                    layer_tile(
                        w1t[e], 0, MC1, KC1,
                        lambda k, xb=xb, T=T: xall[:, xb + k * T:
                                                   xb + (k + 1) * T],
                        lambda m, w, hb=hb, T=T: hall[:, hb + m * T:
                                                      hb + (m + w) * T],
                        T, True, e, 0, singles=True)
                    continue
                emit_l1(i, e, T)
                if i >= 2:
                    ee, tt, cco, TT, _, _ = seq[i - 2]
                    emit_l2(i - 2, ee, TT)
                if i >= 4:
                    ee, tt, cco, TT, _, _ = seq[i - 4]
                    emit_l3(i - 4, ee, TT)
            for e in range(min(2, E)):
                w4t[e] = wp1.tile([128, MC4 * KC4 * 128], f16, tag="w1",
                                  name="w4t")
                nc.gpsimd.dma_start(w4t[e][:], w4[e])
            catchup = ([("l2", i) for i in range(max(0, n_tiles - 2),
                                                  n_tiles)] +
                       [("l3", i) for i in range(max(0, n_tiles - 4),
                                                 n_tiles)])

            # -------- region B: L3 leading L4 by 2 tiles --------
            yring = [0]
            YENGS = (nc.sync,)

            def emit_l4(si, e, T):
                last = si == n_tiles - 1
                yo = seq[si][5]
                db = hco[si]
                yap = yt[yo:yo + 128 * MC4 * T].rearrange("(p f) -> p f",
                                                          p=128)
                half = MC4 // 2
                ytl0 = yp.tile([128, half * T], f16, tag="y", name="ytl0")
                ytl1 = yp.tile([128, half * T], f16, tag="y", name="ytl1")

                def on_chunk(m, ytl0=ytl0, ytl1=ytl1, yap=yap, half=half,
                             T=T, last=last, si=si):
                    if last:
                        # store per evicted pair across three idle rings; the
                        # final pair goes out in two half-pair stores so the
                        # tail waits on ~64KB, not 512KB
                        if m % 2 == 1:
                            p = m - 1
                            ytl = ytl0 if p < half else ytl1
                            mm = p % half
                            if p + 2 < MC4:
                                nc.sync.dma_start(yap[:, p * T:(p + 2) * T],
                                                  ytl[:, mm * T:(mm + 2) * T])
                            else:
                                nc.sync.dma_start(
                                    yap[:, p * T:(p + 1) * T],
                                    ytl[:, mm * T:(mm + 1) * T])
                                nc.scalar.dma_start(
                                    yap[:, (p + 1) * T:(p + 2) * T],
                                    ytl[:, (mm + 1) * T:(mm + 2) * T])
                        return
                    # per-tile halves ride the sync HWDGE ring
                    if m == half - 1:
                        nc.sync.dma_start(yap[:, :half * T], ytl0[:])
                    elif m == MC4 - 1:
                        nc.sync.dma_start(yap[:, half * T:], ytl1[:])

                def ydst(m, w, ytl0=ytl0, ytl1=ytl1, half=half, T=T):
                    ytl = ytl0 if m < half else ytl1
                    mm = m % half
                    return ytl[:, mm * T:(mm + w) * T]

                layer_tile(w4t[e], 0, MC4, KC4,
                           lambda k, db=db, T=T: dall[:, db + k * T:
                                                      db + (k + 1) * T],
                           ydst, T, False, e, 3, on_chunk=on_chunk)

                # stream the next w4 once this expert's tiles are done
                if (si + 1 == n_tiles or seq[si + 1][0] != e):
                    enext = e + 2
                    if enext < E and tiles[enext][1]:
                        w4t[enext] = wp1.tile([128, MC4 * KC4 * 128], f16,
                                              tag="w1", name="w4t")
                        eng = nc.sync if enext == 2 else nc.gpsimd
                        eng.dma_start(w4t[enext][:], w4[enext])

            # leftover L2/L3s are spread between the first L4 tiles
            # (independent experts, ample scalar slack)
            for i, (e, t, co, T, xo, yo) in enumerate(seq):
                emit_l4(i, e, T)
                if catchup:
                    what, j = catchup.pop(0)
                    ee, tt, cco, TT, _, _ = seq[j]
                    (emit_l2 if what == "l2" else emit_l3)(j, ee, TT)

    nc.compile()
    return nc


def kernel(**inputs) -> np.ndarray:
    from concourse.bass_utils import run_bass_kernel_spmd

    X = np.ascontiguousarray(inputs["X"], dtype=np.float32)
    sym_ids = np.asarray(inputs["sym_ids"]).astype(np.int64).ravel()
    We = [inputs["We1"], inputs["We2"], inputs["Wd1"], inputs["Wd2"]]
    be = [np.asarray(inputs["be1"], dtype=np.float32),
          np.asarray(inputs["be2"], dtype=np.float32),
          np.asarray(inputs["bd1"], dtype=np.float32),
          np.asarray(inputs["bd2"], dtype=np.float32)]

    N, D_IN = X.shape
    E, _, H1 = We[0].shape
    LAT = We[1].shape[2]
    D_OUT = We[3].shape[2]
    KC1 = D_IN // 128
    MC4 = D_OUT // 128
    use_bias = any(np.any(b) for b in be)

    # ---- host routing: per-expert, per-core index assignment ----
    core_idx = [[None] * E for _ in range(N_CORES)]
    C_e = [0] * E
    for e in range(E):
        idx = np.flatnonzero(sym_ids == e)
        n = len(idx)
        base, rem = divmod(n, N_CORES)
        s = 0
        for c in range(N_CORES):
            cnt = base + (1 if c < rem else 0)
            core_idx[c][e] = idx[s:s + cnt]
            s += cnt
        C_e[e] = -(-(base + (1 if rem else 0)) // 8) * 8  # pad to mult of 8

    # per-expert tiling: widths in [128,512]; the first expert leads with its
    # narrowest tile (fast start on minimal data), the last expert ends with
    # its narrowest (minimal exposed tail)
    tiles = []
    off = 0
    first_e = next((e for e in range(E) if C_e[e]), 0)
    last_e = next((e for e in reversed(range(E)) if C_e[e]), E - 1)
    for e in range(E):
        if e == last_e and 1024 < C_e[e] <= 1280:
            # end on a 256-wide tile: shortest exposed tail chain
            ws = [512, C_e[e] - 768, 256]
        elif e == first_e and C_e[e] > 512 + 256:
            # lead with a 256-wide tile: minimal startup-critical x DMA
            ws = [256] + sorted(_tile_widths(C_e[e] - 256))
        else:
            ws = _tile_widths(C_e[e])
            if e == first_e:
                ws.sort()
            if e == last_e:
                ws.sort(reverse=True)
        tiles.append((off, tuple(ws)))
        off += sum(ws)
    C_tot = off

    # ---- build / fetch compiled program ----
    dims = (D_IN, H1, LAT, D_OUT, E, C_tot)
    n_bias_cols = E * (H1 + LAT + H1 + D_OUT) // 128
    key = (dims, tuple(tiles), use_bias, _EVICT_MODE)
    nc = _PROGRAM_CACHE.get(key)
    if nc is None:
        nc = _build_program(dims, tiles, use_bias, n_bias_cols)
        _PROGRAM_CACHE[key] = nc

    # ---- prepare inputs ----
    XrT = np.ascontiguousarray(X.astype(np.float16).T)       # [D_IN, N]
    XrT_z = np.concatenate(
        [XrT, np.zeros((D_IN, 1), np.float16)], axis=1)      # pad col = N

    # weights in device layout: [E, 128, MC*KC*128] (m-major blocks)
    def wdev(w, kc, mw):
        mc = mw // 128
        return np.ascontiguousarray(
            w.astype(np.float32).astype(np.float16)
            .reshape(E, kc, 128, mc, 128).transpose(0, 2, 3, 1, 4)
            .reshape(E, 128, mc * kc * 128))

    Wd1 = wdev(We[0], KC1, H1)
    Wd2 = wdev(We[1], H1 // 128, LAT)
    Wd3 = wdev(We[2], LAT // 128, H1)
    Wd23 = np.concatenate([Wd2, Wd3], axis=2)
    Wd4 = wdev(We[3], H1 // 128, D_OUT)

    seq, x_total, y_total = _plan(dims, tiles)

    bias_h = None
    if use_bias:
        bias_h = np.zeros((128, n_bias_cols), np.float32)
        col = 0
        for e in range(E):
            for b in (be[0][e], be[1][e], be[2][e], be[3][e]):
                for mch in range(len(b) // 128):
                    bias_h[:, col] = b[mch * 128:(mch + 1) * 128]
                    col += 1

    perms = []
    in_maps = []
    for c in range(N_CORES):
        perm = np.full(C_tot, N, dtype=np.int64)
        for e in range(E):
            o = tiles[e][0]
            idx = core_idx[c][e]
            perm[o:o + len(idx)] = idx
        perms.append(perm)
        g3 = XrT_z[:, perm].reshape(KC1, 128, C_tot)
        xflat = np.empty(x_total, dtype=np.float16)
        for e, t, co, T, xo, yo in seq:
            xflat[xo:xo + 128 * KC1 * T] = (
                g3[:, :, co:co + T].transpose(1, 0, 2).reshape(-1))
        m = {"xt": xflat, "w1": Wd1, "w23": Wd23, "w4": Wd4}
        if use_bias:
            m["bias"] = bias_h
        in_maps.append(m)

    res = run_bass_kernel_spmd(nc, in_maps, core_ids=list(range(N_CORES)),
                               trace=_TRACE)
    _LAST["res"] = res

    # ---- unshard ----
    Y = np.empty((N, D_OUT), dtype=np.float32)
    for c in range(N_CORES):
        yflat = res.results[c]["yt"]
        ytc = np.empty((D_OUT, C_tot), dtype=np.float32)
        for e, t, co, T, xo, yo in seq:
            ytc[:, co:co + T] = (
                yflat[yo:yo + 128 * MC4 * T].astype(np.float32)
                .reshape(128, MC4, T).transpose(1, 0, 2).reshape(D_OUT, T))
        perm = perms[c]
        valid = perm != N
        Y[perm[valid]] = ytc.T[valid]
    return Y

